# revision 49
# baseline (speedup 1.0000x reference)
"""BiRWKV layer kernel for 8 Trainium2 NeuronCores.

Strategy (data-parallel over B=8, one batch element per core):
  - (channel, time) layout on chip: channels on the 128 SBUF partitions
    (C=512 -> 4 blocks), time on the free dim. Full-T (4096) stripe
    arrays per (direction, channel-block).
  - r/k/v projections are bf16 matmuls (lhsT = W block, rhs = x^T block)
    accumulated over 4 input-channel blocks into PSUM (fp32); x^T is
    fully SBUF-resident so weight blocks are reused across 2-wide
    time-tile groups.
  - WKV runs UNSTABILIZED (mathematically equal to the reference's
    log-sum-exp form; values stay in range since |w|*T <= ~28, k~N(0,1)):
        den_t = d*den_{t-1} + e^{k_t};  num_t = d*num_{t-1} + e^{k_t} v_t
        y_t   = (num_{t-1} + E e^{k_t} v_t) / (den_{t-1} + E e^{k_t}),
    with E = e^u folded in as a per-partition scalar_tensor_tensor
    scalar (no exp(k+u) activation needed).
  - den/num recurrences: DVE tensor_tensor_scan, 2 chained 2048-wide
    instructions per variable sharing one (128, 4097) chain tile (the
    second half's init reads the first half's last element in place).
    Backward direction uses reversed access patterns.
  - y = (num_prev + ekb) / (den_prev + ekb*v...) with ekb = e^{k+u} from
    an Exp ACT with per-partition bias; ekv = ekbv * e^{-u} via a
    Copy-ACT with per-partition scale. dy/ny adds and ekbv mult run on
    GpSimd (SBUF-only tensor_tensor); the divide is vector.reciprocal
    (bf16) + three 2x-mode DVE tensor_tensor ops for q, q*th, and
    y = q + q*th. 0.5 of the sigmoid is folded into W_out on the host.
  - Scalar engine only runs Exp/Tanh/Copy (one ACT table, no reloads).
  - Output projection keeps W_out blocks as matmul lhsT (stationary)
    and y tiles (channel, time) as rhs; result is (C, T) in PSUM ->
    SBUF -> HBM, transposed to (T, C) on the host.
"""

import numpy as np
import ml_dtypes

B, T, C = 8, 4096, 512
TT = 512           # time tile (psum width)
NTT = T // TT      # 8
CB = 4             # channel blocks
HW = 2048          # scan half width
CHK = 2048         # y-stage chunk width
NCHK = T // CHK    # 2

_CACHE = {}


def _apply_tile_patches():
    """walrus in this container rejects instructions with >1 sync wait
    ("Too many sync wait commands"). Split excess waits onto same-engine
    nop carriers, and do the same for the TileContext tail drain."""
    import concourse.tile as tile_mod
    from concourse import mybir
    from concourse.vector_clock import ScopedClock

    if getattr(tile_mod, "_wait_split_patched", False):
        return
    MAXW = 1

    _orig_add = tile_mod.TileContext._add_instruction

    def _split_add(self, inst):
        si = inst.sync_info
        if si is not None and si.on_wait and len(si.on_wait) > MAXW:
            waits = list(si.on_wait)
            k = 0
            while len(waits) > MAXW:
                chunk, waits = waits[:MAXW], waits[MAXW:]
                carrier = mybir.InstNoOp(
                    name=f"{inst.name}_wsplit{k}",
                    engine=inst.engine,
                    bass_nofuse=True,
                    sync_info=mybir.SyncInfo(on_wait=chunk, on_update=[]),
                )
                k += 1
                _orig_add(self, carrier)
            inst.sync_info = mybir.SyncInfo(
                on_wait=waits, on_update=list(si.on_update)
            )
        return _orig_add(self, inst)

    def _drain_and_barrier(self, tick_clock, wait_clock):
        drain_inst = self.nc.sync.drain()
        wait_clock.add_sem_waits(
            drain_inst.ins, ScopedClock({None: tick_clock.global_clock})
        )
        si = drain_inst.ins.sync_info
        if si is not None and si.on_wait and len(si.on_wait) > MAXW:
            waits = list(si.on_wait)
            drain_inst.ins.sync_info = mybir.SyncInfo(
                on_wait=waits[:MAXW], on_update=list(si.on_update)
            )
            rest = waits[MAXW:]
            while rest:
                chunk, rest = rest[:MAXW], rest[MAXW:]
                n = self.nc.sync.nop(nofuse=True)
                n.ins.sync_info = mybir.SyncInfo(on_wait=chunk, on_update=[])

        self.nc.all_engine_barrier()
        assert self.sems is not None
        popped = self.nc._tile_sem_poison_stack.pop()
        assert popped is self._sem_poison
        self.nc.clear_and_free_semaphores(list(self.sems.allocated().values()))
        self.nc.all_engine_barrier()

    tile_mod.TileContext._add_instruction = _split_add
    tile_mod.TileContext._drain_and_barrier = _drain_and_barrier
    tile_mod._wait_split_patched = True


def _build_nc():
    import concourse.bass as bass
    import concourse.tile as tile
    from concourse import mybir

    _apply_tile_patches()

    f32 = mybir.dt.float32
    bf16 = mybir.dt.bfloat16
    Alu = mybir.AluOpType
    Act = mybir.ActivationFunctionType

    nc = bass.Bass()
    from concourse.bass import _add_dep_helper

    xT = nc.dram_tensor("xT", [C, T], bf16, kind="ExternalInput")
    wnames = ["w_rf", "w_kf", "w_vf", "w_rb", "w_kb", "w_vb"]
    wdram = {
        n: nc.dram_tensor(n, [128, 4 * C], bf16, kind="ExternalInput")
        for n in wnames
    }
    wout_d = nc.dram_tensor("wout", [128, 8 * C], bf16, kind="ExternalInput")
    u_f_d = nc.dram_tensor("u_f", [C, 1], f32, kind="ExternalInput")
    u_b_d = nc.dram_tensor("u_b", [C, 1], f32, kind="ExternalInput")
    eu_f_d = nc.dram_tensor("eu_f", [C, 1], f32, kind="ExternalInput")
    eu_b_d = nc.dram_tensor("eu_b", [C, 1], f32, kind="ExternalInput")
    dec_f_d = nc.dram_tensor("dec_f", [C, 1], f32, kind="ExternalInput")
    dec_b_d = nc.dram_tensor("dec_b", [C, 1], f32, kind="ExternalInput")
    out_d = nc.dram_tensor("yT", [C, T], f32, kind="ExternalOutput")
    yst = {d: nc.dram_tensor(f"yst_{d}", [C, T], bf16) for d in ("f", "b")}

    # program-order chain for Scalar ACTs so exp/tanh batches and ln/exp
    # batches don't interleave (each interleave costs a 1.28us table load)
    act_state = {"last": None}

    def act(*args, **kwargs):
        i = nc.scalar.activation(*args, **kwargs)
        if act_state["last"] is not None:
            _add_dep_helper(i.ins, act_state["last"], False,
                            "ACT table-set program order")
        act_state["last"] = i.ins
        return i

    def act_copy(out, in_):
        i = nc.scalar.copy(out, in_)
        if act_state["last"] is not None:
            _add_dep_helper(i.ins, act_state["last"], False,
                            "ACT table-set program order")
        act_state["last"] = i.ins
        return i

    with tile.TileContext(nc) as tc:
        with (
            tc.tile_pool(name="wp", bufs=1) as wp,
            tc.tile_pool(name="cst", bufs=1) as cst,
            tc.tile_pool(name="xr", bufs=1) as xrp,
            tc.tile_pool(name="arr", bufs=3) as arrp,
            tc.tile_pool(name="chn", bufs=2) as chnp,
            tc.tile_pool(name="yc", bufs=2) as ycp,
            tc.tile_pool(name="op", bufs=2) as opp,
            tc.tile_pool(name="ps", bufs=1, space="PSUM") as psp,
        ):
            # ---- resident weights, x, constants ----
            wout = wp.tile([128, 8 * C], bf16, name="wout")
            nc.sync.dma_start(wout[:], wout_d[:])
            wt = {}
            for n in wnames:
                wt[n] = wp.tile([128, 4 * C], bf16, tag=n, name=n)
                nc.sync.dma_start(wt[n][:], wdram[n][:])
            xts = {}
            for kb in range(4):
                xts[kb] = xrp.tile([128, T], bf16, tag=f"x{kb}", name=f"x{kb}")
                for q in range(4):
                    qs = slice(q * (T // 4), (q + 1) * (T // 4))
                    nc.sync.dma_start(xts[kb][:, qs],
                                      xT[kb * 128:(kb + 1) * 128, qs])
            u_t, eu_t, dec_t = {}, {}, {}
            for cb in range(CB):
                sl = slice(cb * 128, (cb + 1) * 128)
                for d, ud, eud, dd in (("f", u_f_d, eu_f_d, dec_f_d),
                                       ("b", u_b_d, eu_b_d, dec_b_d)):
                    u_t[(d, cb)] = cst.tile([128, 1], f32, tag=f"u{d}{cb}",
                                            name=f"u{d}{cb}")
                    nc.sync.dma_start(u_t[(d, cb)][:], ud[sl, :])
                    eu_t[(d, cb)] = cst.tile([128, 1], f32, tag=f"e{d}{cb}",
                                             name=f"e{d}{cb}")
                    nc.sync.dma_start(eu_t[(d, cb)][:], eud[sl, :])
                    dec_t[(d, cb)] = cst.tile([128, 1], f32, tag=f"d{d}{cb}",
                                              name=f"d{d}{cb}")
                    nc.sync.dma_start(dec_t[(d, cb)][:], dd[sl, :])

            # carry tiles: scan state handed from a (d, cb) stripe's first
            # time-segment to its second
            carry = {}
            for d in ("f", "b"):
                for cb in range(CB):
                    for v in ("d", "n"):
                        carry[(d, cb, v)] = cst.tile(
                            [128, 1], bf16, tag=f"c{d}{cb}{v}",
                            name=f"c{d}{cb}{v}")

            # ---- half-stripe front: projections + ACTs + ekv + scans for
            # one (direction, channel block, time half)
            def half_front(d, cb, seg):
                fwd = d == "f"
                first = (seg == 0) if fwd else (seg == 1)
                tb = seg * HW
                wr, wk, wv = wt["w_r" + d], wt["w_k" + d], wt["w_v" + d]
                ek = arrp.tile([128, HW], bf16, tag="ek", name="ek")
                er = arrp.tile([128, HW], bf16, tag="er", name="er")
                ekv = arrp.tile([128, HW], bf16, tag="ekv", name="ekv")
                chd = chnp.tile([128, HW + 1], bf16, tag="chd", name="chd")
                chn = chnp.tile([128, HW + 1], bf16, tag="chn", name="chn")
                eu = eu_t[(d, cb)][:, 0:1]

                # ekv = ek * v is a DVE mult straight out of psum_v
                for ttg in range(2):
                    for cls, w in (("k", wk), ("v", wv), ("r", wr)):
                        pss = {}
                        for h in range(2):
                            pss[h] = psp.tile([128, TT], f32, tag=f"p{cls}",
                                              bufs=(3 if cls == "v" else 2),
                                              name=f"p{cls}")
                        for kb in range(4):
                            wsl = w[:, kb * C + cb * 128:
                                    kb * C + cb * 128 + 128]
                            for h in range(2):
                                tg = tb + (2 * ttg + h) * TT
                                nc.tensor.matmul(
                                    pss[h][:], wsl, xts[kb][:, tg:tg + TT],
                                    start=(kb == 0), stop=(kb == 3))
                        for h in range(2):
                            tl = (2 * ttg + h) * TT
                            if cls == "k":
                                nc.scalar.activation(ek[:, tl:tl + TT],
                                                     pss[h][:], Act.Exp)
                            elif cls == "v":
                                # ekv reads psum_v directly (PSUM ports are
                                # contention-free for the DVE); pv runs 3
                                # banks deep so PE isn't gated on the drain
                                nc.vector.tensor_mul(ekv[:, tl:tl + TT],
                                                     ek[:, tl:tl + TT],
                                                     pss[h][:])
                            else:
                                # er2 = 1 + e^{-r}; both ACTs in the
                                # natural_log_exp table, second in place
                                nc.scalar.activation(er[:, tl:tl + TT],
                                                     pss[h][:], Act.Exp,
                                                     scale=-1.0)
                                nc.scalar.activation(er[:, tl:tl + TT],
                                                     er[:, tl:tl + TT],
                                                     Act.Copy, bias=1.0)

                # one 2048-wide scan per variable; carry chains the halves
                decbc = dec_t[(d, cb)][:, 0:1].broadcast_to([128, HW])
                if fwd:
                    for ch, arr, v in ((chd, ek, "d"), (chn, ekv, "n")):
                        if first:
                            nc.vector.memset(ch[:, 0:1], 0.0)
                        else:
                            nc.vector.tensor_copy(ch[:, 0:1],
                                                  carry[(d, cb, v)][:])
                        nc.vector.tensor_tensor_scan(
                            ch[:, 1:HW + 1], decbc, arr[:],
                            ch[:, 0:1], Alu.mult, Alu.add)
                        if first:
                            nc.vector.tensor_copy(carry[(d, cb, v)][:],
                                                  ch[:, HW:HW + 1])
                    den_prev = chd[:, 0:HW]
                    num_prev = chn[:, 0:HW]
                else:
                    for ch, arr, v in ((chd, ek, "d"), (chn, ekv, "n")):
                        if first:
                            nc.vector.memset(ch[:, HW:HW + 1], 0.0)
                        else:
                            nc.vector.tensor_copy(ch[:, HW:HW + 1],
                                                  carry[(d, cb, v)][:])
                        nc.vector.tensor_tensor_scan(
                            ch[:, 0:HW][:, ::-1], decbc, arr[:][:, ::-1],
                            ch[:, HW:HW + 1], Alu.mult, Alu.add)
                        if first:
                            nc.vector.tensor_copy(carry[(d, cb, v)][:],
                                                  ch[:, 0:1])
                    den_prev = chd[:, 1:HW + 1]
                    num_prev = chn[:, 1:HW + 1]
                return dict(d=d, cb=cb, tb=tb, ek=ek, er=er, ekv=ekv,
                            den_prev=den_prev, num_prev=num_prev, eu=eu)

            # ---- y stage, one 2048 chunk per half-stripe
            # The sigmoid folds into the division:
            #   y = sig(r) * ny / dy = ny / (dy * (1 + e^{-r}))
            # and the e^u boost folds into the adds as an STT scalar:
            #   dy = (ek * e^u) + den_prev,  ny = (ekv * e^u) + num_prev
            # so the y-stage is 2 STTs, one GP mult, a ln/exp pair, and a
            # final mult; every Scalar ACT in the kernel uses the single
            # natural_log_exp table (zero table reloads). Dead array
            # slices are recycled: inv->ek, y->er.
            def half_y(st):
                d, cb, tb = st["d"], st["cb"], st["tb"]
                ek, er, ekv = st["ek"], st["er"], st["ekv"]
                den_prev, num_prev, eu = (st["den_prev"], st["num_prev"],
                                          st["eu"])
                dy = ycp.tile([128, CHK], bf16, tag="dy", name="dy")
                dy2 = ycp.tile([128, CHK], bf16, tag="dy2", name="dy2")
                ny = ycp.tile([128, CHK], bf16, tag="ny", name="ny")
                lnb = ycp.tile([128, CHK], f32, tag="lnb", bufs=1,
                               name="lnb")
                inv = ek[:, :]
                y = er[:, :]
                nc.vector.scalar_tensor_tensor(
                    dy[:], ek[:, :], eu, den_prev, Alu.mult, Alu.add)
                nc.gpsimd.tensor_mul(dy2[:], dy[:], er[:, :])
                nc.scalar.activation(lnb[:], dy2[:], Act.Ln)
                nc.scalar.activation(inv, lnb[:], Act.Exp, scale=-1.0)
                nc.vector.scalar_tensor_tensor(
                    ny[:], ekv[:, :], eu, num_prev, Alu.mult, Alu.add)
                nc.gpsimd.tensor_mul(y, ny[:], inv)
                nc.sync.dma_start(
                    yst[d][cb * 128:(cb + 1) * 128, tb:tb + HW], y)

            # ---- output projection for one time half (needs both dirs)
            def out_proj(seg):
                for tc_i in range(seg * 4, seg * 4 + 4):
                    t0 = tc_i * TT
                    yld = {}
                    for j in range(8):
                        dd = "f" if j < 4 else "b"
                        cbj = j % 4
                        yld[j] = opp.tile([128, TT], bf16, tag=f"yl{j}",
                                          name=f"yl{j}")
                        nc.sync.dma_start(
                            yld[j][:],
                            yst[dd][cbj * 128:(cbj + 1) * 128, t0:t0 + TT])
                    for cbo in range(CB):
                        pso = psp.tile([128, TT], f32, tag="po", bufs=1,
                                       name="pso")
                        for j in range(8):
                            blk = j * 4 + cbo
                            nc.tensor.matmul(
                                pso[:], wout[:, blk * 128:(blk + 1) * 128],
                                yld[j][:], start=(j == 0), stop=(j == 7))
                        osb = opp.tile([128, TT], f32, tag="osb",
                                       name="osb")
                        nc.vector.tensor_copy(osb[:], pso[:])
                        nc.sync.dma_start(
                            out_d[cbo * 128:(cbo + 1) * 128, t0:t0 + TT],
                            osb[:])

            # Phase order (f-low, b-high, f-high, b-low) lets the high
            # half's output projection overlap the final b-low phase.
            # Emission is software-pipelined with a 1-half-stripe lag so
            # each engine alternates between consecutive half-stripes.
            plan = ([("f", cb, 0) for cb in range(CB)] +
                    [("b", cb, 1) for cb in range(CB)] +
                    [("f", cb, 1) for cb in range(CB)] +
                    [("b", cb, 0) for cb in range(CB)])
            pending = None
            for i, (d, cb, seg) in enumerate(plan):
                st = half_front(d, cb, seg)
                if pending is not None:
                    half_y(pending)
                    if i == 12:
                        out_proj(1)   # f-high + b-high y complete
                pending = st
            half_y(pending)
            out_proj(0)

    return nc


def _host_prep(x, W_rkv, W_out, time_decay, time_first, time_decay_rev,
               time_first_rev):
    bf16 = ml_dtypes.bfloat16
    f32 = np.float32

    Wr = W_rkv.reshape(C, 2, 3, C)
    pieces = {
        "w_rf": Wr[:, 0, 0], "w_kf": Wr[:, 0, 1], "w_vf": Wr[:, 0, 2],
        "w_rb": Wr[:, 1, 0], "w_kb": Wr[:, 1, 1], "w_vb": Wr[:, 1, 2],
    }
    wmaps = {}
    for n, p in pieces.items():
        wmaps[n] = np.ascontiguousarray(
            p.reshape(4, 128, C).transpose(1, 0, 2).reshape(128, 4 * C)
        ).astype(bf16)

    # W_out blocks as stationary lhsT: block (j, cbo) = W_out rows
    # j*128:(j+1)*128, cols cbo*128:(cbo+1)*128, laid out at free offset
    # (j*4 + cbo)*128.
    Wo = W_out.reshape(8, 128, 4, 128)                  # (j, jp, cbo, cp)
    Wo = Wo.transpose(1, 0, 2, 3).reshape(128, 32 * 128)  # (jp, j*4+cbo, cp)
    wout = np.ascontiguousarray(Wo).astype(bf16)

    u_f = np.ascontiguousarray(time_first.reshape(C, 1)).astype(f32)
    u_b = np.ascontiguousarray(time_first_rev.reshape(C, 1)).astype(f32)
    eu_f = np.exp(time_first.astype(np.float64)).reshape(C, 1).astype(f32)
    eu_b = np.exp(time_first_rev.astype(np.float64)).reshape(C, 1).astype(f32)
    dec_f = np.exp(-np.exp(time_decay.astype(np.float64))).reshape(C, 1).astype(f32)
    dec_b = np.exp(-np.exp(time_decay_rev.astype(np.float64))).reshape(C, 1).astype(f32)

    shared = dict(wout=wout, u_f=u_f, u_b=u_b, eu_f=eu_f, eu_b=eu_b,
                  dec_f=dec_f, dec_b=dec_b, **wmaps)
    in_maps = []
    for b in range(B):
        m = dict(shared)
        m["xT"] = np.ascontiguousarray(x[b].T).astype(bf16)
        in_maps.append(m)
    return in_maps


def kernel(x, W_rkv, W_out, time_decay, time_first, time_decay_rev,
           time_first_rev, _trace=False):
    from concourse.bass_utils import run_bass_kernel_spmd

    x = np.asarray(x, dtype=np.float32)
    W_rkv = np.asarray(W_rkv, dtype=np.float32)
    W_out = np.asarray(W_out, dtype=np.float32)
    time_decay = np.asarray(time_decay, dtype=np.float32)
    time_first = np.asarray(time_first, dtype=np.float32)
    time_decay_rev = np.asarray(time_decay_rev, dtype=np.float32)
    time_first_rev = np.asarray(time_first_rev, dtype=np.float32)

    if "nc" not in _CACHE:
        _CACHE["nc"] = _build_nc()
    nc = _CACHE["nc"]

    in_maps = _host_prep(x, W_rkv, W_out, time_decay, time_first,
                         time_decay_rev, time_first_rev)
    res = run_bass_kernel_spmd(
        nc, in_maps, core_ids=list(range(B)), trace=_trace
    )
    _CACHE["last_result"] = res
    out = np.stack([
        np.ascontiguousarray(res.results[b]["yT"].astype(np.float32).T)
        for b in range(B)
    ])
    return out
